# revision 24
# baseline (speedup 1.0000x reference)
"""Cross-attention kernel for Trainium2, sharded over 8 NeuronCores.

Problem (hardcoded): b=4, n=m=2048, query_dim=context_dim=512,
heads=8, dim_head=64 (inner=512), f32 I/O.

Sharding: data-parallel over (batch, query-half): core c -> batch c//2,
query rows [(c%2)*1024, (c%2+1)*1024). Each core holds the full K/V
context for its batch, so there are no collectives and output shards
tile the full output exactly.

Layout strategy (all matmul inputs bf16, accumulation f32 in PSUM):
  - Host pre-transposes activations: pixelT [512c, 1024n], patchT [512c, 2048m].
  - Q^T [inner, n] and K^T [inner, m] computed with weights as stationary.
  - V kept natural [m, inner], stored per m-chunk as [128, 8 heads, 65]
    with a constant-1 column appended per head: the attention-output
    matmul then yields [65, n] per head where row 64 = sum(exp(scores)),
    giving the softmax denominator for free.
  - scores^T [m, n] per head via k=64 matmuls (kT stationary, qT moving);
    exp runs on ScalarE directly PSUM->SBUF(bf16) with scale=1/8 folded in.
  - attn-out^T [65, n] accumulates over 16 m-chunks in PSUM; normalized by
    reciprocal of row 64 (broadcast across partitions via gpsimd).
  - Final projection: outT (inner on partitions) stationary, Wo moving;
    bias added on the PSUM->SBUF copy.
"""

import numpy as np
import ml_dtypes

import concourse.bass as bass
import concourse.mybir as mybir
import concourse.tile as tile
from concourse import bacc
from concourse.bass_utils import run_bass_kernel_spmd

BF16 = mybir.dt.bfloat16
F32 = mybir.dt.float32

B, N, M = 4, 2048, 2048
CDIM, INNER = 512, 512
H, D = 8, 64
NSH = N // 2  # query rows per core
N_CORES = 8
SCALE = D ** -0.5

CC = CDIM // 128   # contraction chunks for projections (4)
IC = INNER // 128  # inner-dim chunks (4)
MT = M // 128      # m tiles (16)
NJ = NSH // 512    # n chunks of 512 (2)
NT = NSH // 128    # n tiles (8)
MJ = M // 512      # m chunks of 512 (4)


def build_nc() -> bass.Bass:
    nc = bacc.Bacc(None)

    pixelT = nc.dram_tensor("pixelT", [CDIM, NSH], BF16, kind="ExternalInput")
    patchT = nc.dram_tensor("patchT", [CDIM, M], BF16, kind="ExternalInput")
    wq = nc.dram_tensor("wq", [CDIM, INNER], BF16, kind="ExternalInput")
    wk = nc.dram_tensor("wk", [CDIM, INNER], BF16, kind="ExternalInput")
    wv = nc.dram_tensor("wv", [CDIM, INNER], BF16, kind="ExternalInput")
    wo = nc.dram_tensor("wo", [INNER, CDIM], BF16, kind="ExternalInput")
    bo = nc.dram_tensor("bo", [CDIM], F32, kind="ExternalInput")
    out = nc.dram_tensor("out", [NSH, CDIM], F32, kind="ExternalOutput")

    with tile.TileContext(nc) as tc:
        with (
            tc.tile_pool(name="weights", bufs=1) as wpool,
            tc.tile_pool(name="acts", bufs=1) as apool,
            tc.tile_pool(name="qkv", bufs=1) as qkvpool,
            tc.tile_pool(name="vsb", bufs=MT) as vpool,
            tc.tile_pool(name="attn", bufs=6) as attnpool,
            tc.tile_pool(name="small", bufs=4) as rpool,
            tc.tile_pool(name="stage", bufs=3) as stpool,
        ):
            # ---- load weights + activations -------------------------------
            wq_sb = wpool.tile([128, CC, INNER], BF16, tag="wq")
            wk_sb = wpool.tile([128, CC, INNER], BF16, tag="wk")
            wv_sb = wpool.tile([128, CC, INNER], BF16, tag="wv")
            wo_sb = wpool.tile([128, IC, CDIM], BF16, tag="wo")
            nc.gpsimd.dma_start(wq_sb, wq.rearrange("(cc p) i -> p cc i", p=128))
            nc.gpsimd.dma_start(wk_sb, wk.rearrange("(cc p) i -> p cc i", p=128))
            nc.gpsimd.dma_start(wv_sb, wv.rearrange("(cc p) i -> p cc i", p=128))
            nc.gpsimd.dma_start(wo_sb, wo.rearrange("(ic p) o -> p ic o", p=128))

            bo_sb = wpool.tile([128, CDIM], F32, tag="bo")
            nc.sync.dma_start(
                bo_sb,
                bass.AP(tensor=bo[:].tensor, offset=0, ap=[[0, 128], [1, CDIM]]),
            )

            pixT = apool.tile([128, CC, NSH], BF16, tag="pixT")
            patT = apool.tile([128, CC, M], BF16, tag="patT")
            pix_r = pixelT.rearrange("(cc p) n -> p cc n", p=128)
            pat_r = patchT.rearrange("(cc p) m -> p cc m", p=128)
            for cc in range(CC):
                nc.sync.dma_start(pixT[:, cc, :], pix_r[:, cc, :])
                nc.sync.dma_start(patT[:, cc, 0:1024], pat_r[:, cc, 0:1024])
                nc.sync.dma_start(patT[:, cc, 1024:2048], pat_r[:, cc, 1024:2048])

            # warm the exp table early so the first real exp isn't gated on it
            warm = rpool.tile([1, 16], BF16, tag="warm")
            nc.scalar.activation(
                warm, bo_sb[0:1, 0:16], mybir.ActivationFunctionType.Exp
            )

            qT = qkvpool.tile([128, IC, NSH], BF16, tag="qT")
            # kTp: per head a full-k=128 stationary — the head's K^T in its own
            # 64-row range, zeros in the other head's rows. Streaming cost of a
            # matmul is N cycles regardless of k, and the full-height stationary
            # keeps the PE activity monitor (HAM) at the 2.4 GHz clock.
            kTp = qkvpool.tile([128, IC, 2, M], BF16, tag="kTp")
            for ic in range(IC):
                nc.vector.memset(kTp[D : 2 * D, ic, 0, :], 0.0)
                nc.vector.memset(kTp[0:D, ic, 1, :], 0.0)
            # v_sb: [m-chunk 128, head, 128] = [V_h | 1 | zeros] — col 64 gives
            # the softmax denominator via the matmul, cols 65..127 pad M to 128.
            v_sb = [
                vpool.tile([128, H, 128], BF16, tag="v", name=f"v{mi}")
                for mi in range(MT)
            ]
            for mi in range(MT):
                nc.vector.memset(v_sb[mi][:, :, D : 2 * D], 0.0)
                nc.vector.memset(v_sb[mi][:, :, D : D + 1], 1.0)

            # ---- projections ---------------------------------------------
            with tc.tile_pool(name="ppsum", bufs=3, space="PSUM") as ppsum:
                # Q^T [inner, n]
                for ic in range(IC):
                    for nj in range(NJ):
                        ps = ppsum.tile([128, 512], F32, tag="p")
                        for cc in range(CC):
                            nc.tensor.matmul(
                                ps,
                                wq_sb[:, cc, ic * 128 : (ic + 1) * 128],
                                pixT[:, cc, nj * 512 : (nj + 1) * 512],
                                start=(cc == 0),
                                stop=(cc == CC - 1),
                            )
                        nc.vector.tensor_copy(qT[:, ic, nj * 512 : (nj + 1) * 512], ps)
                # K^T [inner, m] -> zero-padded per-head stationaries
                for ic in range(IC):
                    for mj in range(MJ):
                        ps = ppsum.tile([128, 512], F32, tag="p")
                        for cc in range(CC):
                            nc.tensor.matmul(
                                ps,
                                wk_sb[:, cc, ic * 128 : (ic + 1) * 128],
                                patT[:, cc, mj * 512 : (mj + 1) * 512],
                                start=(cc == 0),
                                stop=(cc == CC - 1),
                            )
                        sl = slice(mj * 512, (mj + 1) * 512)
                        nc.vector.tensor_copy(kTp[0:D, ic, 0, sl], ps[0:D, :])
                        nc.vector.tensor_copy(
                            kTp[D : 2 * D, ic, 1, sl], ps[D : 2 * D, :]
                        )
                # V natural [m, inner] -> per-m-chunk [128, H, D+1] with ones col
                for mi in range(MT):
                    ps = ppsum.tile([128, 512], F32, tag="p")
                    for cc in range(CC):
                        nc.tensor.matmul(
                            ps,
                            patT[:, cc, mi * 128 : (mi + 1) * 128],
                            wv_sb[:, cc, :],
                            start=(cc == 0),
                            stop=(cc == CC - 1),
                        )
                    nc.vector.tensor_copy(
                        v_sb[mi][:, :, 0:D], ps.rearrange("p (h d) -> p h d", h=H)
                    )
                    nc.vector.memset(v_sb[mi][:, :, D : D + 1], 1.0)

            # ---- attention (per head) ------------------------------------
            outT = qkvpool.tile([128, IC, NSH], BF16, tag="outT")
            with (
                tc.tile_pool(name="spsum", bufs=3, space="PSUM") as spsum,
                tc.tile_pool(name="opsum", bufs=2, space="PSUM") as opsum,
            ):
                for h in range(H):
                    ic = h // 2
                    po = (h % 2) * D
                    o_ps = [
                        opsum.tile([128, 512], F32, tag="o", name=f"o{h}_{nj}")
                        for nj in range(NJ)
                    ]
                    for mi in range(MT):
                        s_ps = spsum.tile([128, NJ * 512], F32, tag="s")
                        for nj in range(NJ):
                            nc.tensor.matmul(
                                s_ps[:, nj * 512 : (nj + 1) * 512],
                                kTp[:, ic, h % 2, mi * 128 : (mi + 1) * 128],
                                qT[:, ic, nj * 512 : (nj + 1) * 512],
                                start=True,
                                stop=True,
                            )
                        at = attnpool.tile([128, NJ * 512], BF16, tag="at")
                        nc.scalar.activation(
                            at, s_ps, mybir.ActivationFunctionType.Exp, scale=SCALE
                        )
                        for nj in range(NJ):
                            nc.tensor.matmul(
                                o_ps[nj],
                                v_sb[mi][:, h, :],
                                at[:, nj * 512 : (nj + 1) * 512],
                                start=(mi == 0),
                                stop=(mi == MT - 1),
                            )
                    for nj in range(NJ):
                        r = rpool.tile([1, 512], F32, tag="r")
                        nc.vector.reciprocal(r, o_ps[nj][D : D + 1, :])
                        r64 = rpool.tile([D, 512], F32, tag="r64")
                        r_ap = r[0:1, :]
                        nc.sync.dma_start(
                            r64,
                            bass.AP(
                                tensor=r_ap.tensor,
                                offset=r_ap.offset,
                                ap=[[512, 1], [0, D], [1, 512]],
                            ),
                        )
                        nc.vector.tensor_mul(
                            outT[po : po + D, ic, nj * 512 : (nj + 1) * 512],
                            o_ps[nj][0:D, :],
                            r64,
                        )

            # ---- output projection ---------------------------------------
            with tc.tile_pool(name="fpsum", bufs=2, space="PSUM") as fpsum:
                for ni in range(NT):
                    ps = fpsum.tile([128, CDIM], F32, tag="f")
                    for ic in range(IC):
                        nc.tensor.matmul(
                            ps,
                            outT[:, ic, ni * 128 : (ni + 1) * 128],
                            wo_sb[:, ic, :],
                            start=(ic == 0),
                            stop=(ic == IC - 1),
                        )
                    st = stpool.tile([128, CDIM], F32, tag="st")
                    nc.vector.tensor_add(st, ps, bo_sb)
                    nc.sync.dma_start(out[ni * 128 : (ni + 1) * 128, :], st)

    nc.finalize()
    return nc


def make_in_maps(pixel_embed, patch_embed, Wq, Wk, Wv, Wo, bo):
    bf = ml_dtypes.bfloat16
    pixel_embed = np.asarray(pixel_embed, dtype=np.float32)
    patch_embed = np.asarray(patch_embed, dtype=np.float32)
    wq = np.asarray(Wq, dtype=np.float32).astype(bf)
    wk = np.asarray(Wk, dtype=np.float32).astype(bf)
    wv = np.asarray(Wv, dtype=np.float32).astype(bf)
    wo = np.asarray(Wo, dtype=np.float32).astype(bf)
    bo = np.asarray(bo, dtype=np.float32)

    in_maps = []
    for core in range(N_CORES):
        bi, half = divmod(core, 2)
        px = pixel_embed[bi, half * NSH : (half + 1) * NSH, :]  # [NSH, CDIM]
        pa = patch_embed[bi]  # [M, CDIM]
        in_maps.append(
            {
                "pixelT": px.T.astype(bf),
                "patchT": pa.T.astype(bf),
                "wq": wq,
                "wk": wk,
                "wv": wv,
                "wo": wo,
                "bo": bo,
            }
        )
    return in_maps


def gather_out(results):
    out = np.empty((B, N, CDIM), np.float32)
    for core in range(N_CORES):
        bi, half = divmod(core, 2)
        out[bi, half * NSH : (half + 1) * NSH, :] = results[core]["out"]
    return out


_NC_CACHE = {}


def kernel(pixel_embed, patch_embed, Wq, Wk, Wv, Wo, bo, **kw):
    if "nc" not in _NC_CACHE:
        _NC_CACHE["nc"] = build_nc()
    nc = _NC_CACHE["nc"]
    in_maps = make_in_maps(pixel_embed, patch_embed, Wq, Wk, Wv, Wo, bo)
    res = run_bass_kernel_spmd(nc, in_maps, core_ids=list(range(N_CORES)), **kw)
    out = gather_out(res.results)
    if kw.get("trace"):
        return out, res
    return out


# revision 25
# speedup vs baseline: 1.3130x; 1.3130x over previous
"""Cross-attention kernel for Trainium2, sharded over 8 NeuronCores.

Problem (hardcoded): b=4, n=m=2048, query_dim=context_dim=512,
heads=8, dim_head=64 (inner=512), f32 I/O.

Sharding: data-parallel over (batch, query-half): core c -> batch c//2,
query rows [(c%2)*1024, (c%2+1)*1024). Each core holds the full K/V
context for its batch, so there are no collectives and output shards
tile the full output exactly.

Layout strategy (all matmul inputs bf16, accumulation f32 in PSUM):
  - Host pre-transposes activations: pixelT [512c, 1024n], patchT [512c, 2048m].
  - Q^T [inner, n] and K^T [inner, m] computed with weights as stationary.
  - V kept natural [m, inner], stored per m-chunk as [128, 8 heads, 65]
    with a constant-1 column appended per head: the attention-output
    matmul then yields [65, n] per head where row 64 = sum(exp(scores)),
    giving the softmax denominator for free.
  - scores^T [m, n] per head via k=64 matmuls (kT stationary, qT moving);
    exp runs on ScalarE directly PSUM->SBUF(bf16) with scale=1/8 folded in.
  - attn-out^T [65, n] accumulates over 16 m-chunks in PSUM; normalized by
    reciprocal of row 64 (broadcast across partitions via gpsimd).
  - Final projection: outT (inner on partitions) stationary, Wo moving;
    bias added on the PSUM->SBUF copy.
"""

import numpy as np
import ml_dtypes

import concourse.bass as bass
import concourse.mybir as mybir
import concourse.tile as tile
from concourse import bacc
from concourse.bass_utils import run_bass_kernel_spmd

BF16 = mybir.dt.bfloat16
F32 = mybir.dt.float32

B, N, M = 4, 2048, 2048
CDIM, INNER = 512, 512
H, D = 8, 64
NSH = N // 2  # query rows per core
N_CORES = 8
SCALE = D ** -0.5

CC = CDIM // 128   # contraction chunks for projections (4)
IC = INNER // 128  # inner-dim chunks (4)
MT = M // 128      # m tiles (16)
NJ = NSH // 512    # n chunks of 512 (2)
NT = NSH // 128    # n tiles (8)
MJ = M // 512      # m chunks of 512 (4)


def build_nc() -> bass.Bass:
    nc = bacc.Bacc(None)

    pixelT = nc.dram_tensor("pixelT", [CDIM, NSH], BF16, kind="ExternalInput")
    patchT = nc.dram_tensor("patchT", [CDIM, M], BF16, kind="ExternalInput")
    wq = nc.dram_tensor("wq", [CDIM, INNER], BF16, kind="ExternalInput")
    wk = nc.dram_tensor("wk", [CDIM, INNER], BF16, kind="ExternalInput")
    wv = nc.dram_tensor("wv", [CDIM, INNER], BF16, kind="ExternalInput")
    wo = nc.dram_tensor("wo", [INNER, CDIM], BF16, kind="ExternalInput")
    bo = nc.dram_tensor("bo", [CDIM], F32, kind="ExternalInput")
    out = nc.dram_tensor("out", [NSH, CDIM], F32, kind="ExternalOutput")

    with tile.TileContext(nc) as tc:
        with (
            tc.tile_pool(name="weights", bufs=1) as wpool,
            tc.tile_pool(name="acts", bufs=1) as apool,
            tc.tile_pool(name="qkv", bufs=1) as qkvpool,
            tc.tile_pool(name="vsb", bufs=MT) as vpool,
            tc.tile_pool(name="attn", bufs=6) as attnpool,
            tc.tile_pool(name="small", bufs=4) as rpool,
            tc.tile_pool(name="stage", bufs=3) as stpool,
        ):
            # ---- load weights + activations -------------------------------
            wq_sb = wpool.tile([128, CC, INNER], BF16, tag="wq")
            wk_sb = wpool.tile([128, CC, INNER], BF16, tag="wk")
            wv_sb = wpool.tile([128, CC, INNER], BF16, tag="wv")
            wo_sb = wpool.tile([128, IC, CDIM], BF16, tag="wo")
            nc.gpsimd.dma_start(wq_sb, wq.rearrange("(cc p) i -> p cc i", p=128))
            nc.gpsimd.dma_start(wk_sb, wk.rearrange("(cc p) i -> p cc i", p=128))
            nc.gpsimd.dma_start(wv_sb, wv.rearrange("(cc p) i -> p cc i", p=128))
            nc.gpsimd.dma_start(wo_sb, wo.rearrange("(ic p) o -> p ic o", p=128))

            bo_sb = wpool.tile([128, CDIM], F32, tag="bo")
            nc.sync.dma_start(
                bo_sb,
                bass.AP(tensor=bo[:].tensor, offset=0, ap=[[0, 128], [1, CDIM]]),
            )

            pixT = apool.tile([128, CC, NSH], BF16, tag="pixT")
            patT = apool.tile([128, CC, M], BF16, tag="patT")
            pix_r = pixelT.rearrange("(cc p) n -> p cc n", p=128)
            pat_r = patchT.rearrange("(cc p) m -> p cc m", p=128)
            for cc in range(CC):
                nc.sync.dma_start(pixT[:, cc, :], pix_r[:, cc, :])
                nc.sync.dma_start(patT[:, cc, 0:1024], pat_r[:, cc, 0:1024])
                nc.sync.dma_start(patT[:, cc, 1024:2048], pat_r[:, cc, 1024:2048])

            # warm the exp table early so the first real exp isn't gated on it
            warm = rpool.tile([1, 16], BF16, tag="warm")
            nc.scalar.activation(
                warm, bo_sb[0:1, 0:16], mybir.ActivationFunctionType.Exp
            )

            qT = qkvpool.tile([128, IC, NSH], BF16, tag="qT")
            # kTp: per head a full-k=128 stationary — the head's K^T in its own
            # 64-row range, zeros in the other head's rows. Streaming cost of a
            # matmul is N cycles regardless of k, and the full-height stationary
            # keeps the PE activity monitor (HAM) at the 2.4 GHz clock.
            kTp = qkvpool.tile([128, IC, 2, M], BF16, tag="kTp")
            for ic in range(IC):
                nc.vector.memset(kTp[D : 2 * D, ic, 0, :], 0.0)
                nc.vector.memset(kTp[0:D, ic, 1, :], 0.0)
            # v_sb: [m-chunk 128, head, 128] = [V_h | 1 | zeros] — col 64 gives
            # the softmax denominator via the matmul, cols 65..127 pad M to 128.
            v_sb = [
                vpool.tile([128, H, 128], BF16, tag="v", name=f"v{mi}")
                for mi in range(MT)
            ]
            for mi in range(MT):
                nc.vector.memset(v_sb[mi][:, :, D : 2 * D], 0.0)
                nc.vector.memset(v_sb[mi][:, :, D : D + 1], 1.0)

            # ---- projections ---------------------------------------------
            with tc.tile_pool(name="ppsum", bufs=3, space="PSUM") as ppsum:
                # Q^T [inner, n]
                for ic in range(IC):
                    for nj in range(NJ):
                        ps = ppsum.tile([128, 512], F32, tag="p")
                        for cc in range(CC):
                            nc.tensor.matmul(
                                ps,
                                wq_sb[:, cc, ic * 128 : (ic + 1) * 128],
                                pixT[:, cc, nj * 512 : (nj + 1) * 512],
                                start=(cc == 0),
                                stop=(cc == CC - 1),
                            )
                        nc.vector.tensor_copy(qT[:, ic, nj * 512 : (nj + 1) * 512], ps)
                # K^T [inner, m] -> zero-padded per-head stationaries
                for ic in range(IC):
                    for mj in range(MJ):
                        ps = ppsum.tile([128, 512], F32, tag="p")
                        for cc in range(CC):
                            nc.tensor.matmul(
                                ps,
                                wk_sb[:, cc, ic * 128 : (ic + 1) * 128],
                                patT[:, cc, mj * 512 : (mj + 1) * 512],
                                start=(cc == 0),
                                stop=(cc == CC - 1),
                            )
                        sl = slice(mj * 512, (mj + 1) * 512)
                        nc.vector.tensor_copy(kTp[0:D, ic, 0, sl], ps[0:D, :])
                        nc.vector.tensor_copy(
                            kTp[D : 2 * D, ic, 1, sl], ps[D : 2 * D, :]
                        )
                # V natural [m, inner] -> per-m-chunk [128, H, D+1] with ones col
                for mi in range(MT):
                    ps = ppsum.tile([128, 512], F32, tag="p")
                    for cc in range(CC):
                        nc.tensor.matmul(
                            ps,
                            patT[:, cc, mi * 128 : (mi + 1) * 128],
                            wv_sb[:, cc, :],
                            start=(cc == 0),
                            stop=(cc == CC - 1),
                        )
                    nc.vector.tensor_copy(
                        v_sb[mi][:, :, 0:D], ps.rearrange("p (h d) -> p h d", h=H)
                    )
                    nc.vector.memset(v_sb[mi][:, :, D : D + 1], 1.0)

            # ---- attention (per head) ------------------------------------
            outT = qkvpool.tile([128, IC, NSH], BF16, tag="outT")
            with (
                tc.tile_pool(name="spsum", bufs=2, space="PSUM") as spsum,
                tc.tile_pool(name="opsum", bufs=4, space="PSUM") as opsum,
            ):
                for h in range(H):
                    ic = h // 2
                    po = (h % 2) * D
                    o_ps = [
                        opsum.tile([128, 512], F32, tag="o", name=f"o{h}_{nj}")
                        for nj in range(NJ)
                    ]
                    for mi in range(MT):
                        s_ps = spsum.tile([128, NJ * 512], F32, tag="s")
                        for nj in range(NJ):
                            nc.tensor.matmul(
                                s_ps[:, nj * 512 : (nj + 1) * 512],
                                kTp[:, ic, h % 2, mi * 128 : (mi + 1) * 128],
                                qT[:, ic, nj * 512 : (nj + 1) * 512],
                                start=True,
                                stop=True,
                            )
                        at = attnpool.tile([128, NJ * 512], BF16, tag="at")
                        nc.scalar.activation(
                            at, s_ps, mybir.ActivationFunctionType.Exp, scale=SCALE
                        )
                        for nj in range(NJ):
                            nc.tensor.matmul(
                                o_ps[nj],
                                v_sb[mi][:, h, :],
                                at[:, nj * 512 : (nj + 1) * 512],
                                start=(mi == 0),
                                stop=(mi == MT - 1),
                            )
                    for nj in range(NJ):
                        r = rpool.tile([1, 512], F32, tag="r")
                        nc.vector.reciprocal(r, o_ps[nj][D : D + 1, :])
                        r64 = rpool.tile([D, 512], F32, tag="r64")
                        r_ap = r[0:1, :]
                        nc.sync.dma_start(
                            r64,
                            bass.AP(
                                tensor=r_ap.tensor,
                                offset=r_ap.offset,
                                ap=[[512, 1], [0, D], [1, 512]],
                            ),
                        )
                        nc.vector.tensor_mul(
                            outT[po : po + D, ic, nj * 512 : (nj + 1) * 512],
                            o_ps[nj][0:D, :],
                            r64,
                        )

            # ---- output projection ---------------------------------------
            with tc.tile_pool(name="fpsum", bufs=2, space="PSUM") as fpsum:
                for ni in range(NT):
                    ps = fpsum.tile([128, CDIM], F32, tag="f")
                    for ic in range(IC):
                        nc.tensor.matmul(
                            ps,
                            outT[:, ic, ni * 128 : (ni + 1) * 128],
                            wo_sb[:, ic, :],
                            start=(ic == 0),
                            stop=(ic == IC - 1),
                        )
                    st = stpool.tile([128, CDIM], F32, tag="st")
                    nc.vector.tensor_add(st, ps, bo_sb)
                    nc.sync.dma_start(out[ni * 128 : (ni + 1) * 128, :], st)

    nc.finalize()
    return nc


def make_in_maps(pixel_embed, patch_embed, Wq, Wk, Wv, Wo, bo):
    bf = ml_dtypes.bfloat16
    pixel_embed = np.asarray(pixel_embed, dtype=np.float32)
    patch_embed = np.asarray(patch_embed, dtype=np.float32)
    wq = np.asarray(Wq, dtype=np.float32).astype(bf)
    wk = np.asarray(Wk, dtype=np.float32).astype(bf)
    wv = np.asarray(Wv, dtype=np.float32).astype(bf)
    wo = np.asarray(Wo, dtype=np.float32).astype(bf)
    bo = np.asarray(bo, dtype=np.float32)

    in_maps = []
    for core in range(N_CORES):
        bi, half = divmod(core, 2)
        px = pixel_embed[bi, half * NSH : (half + 1) * NSH, :]  # [NSH, CDIM]
        pa = patch_embed[bi]  # [M, CDIM]
        in_maps.append(
            {
                "pixelT": px.T.astype(bf),
                "patchT": pa.T.astype(bf),
                "wq": wq,
                "wk": wk,
                "wv": wv,
                "wo": wo,
                "bo": bo,
            }
        )
    return in_maps


def gather_out(results):
    out = np.empty((B, N, CDIM), np.float32)
    for core in range(N_CORES):
        bi, half = divmod(core, 2)
        out[bi, half * NSH : (half + 1) * NSH, :] = results[core]["out"]
    return out


_NC_CACHE = {}


def kernel(pixel_embed, patch_embed, Wq, Wk, Wv, Wo, bo, **kw):
    if "nc" not in _NC_CACHE:
        _NC_CACHE["nc"] = build_nc()
    nc = _NC_CACHE["nc"]
    in_maps = make_in_maps(pixel_embed, patch_embed, Wq, Wk, Wv, Wo, bo)
    res = run_bass_kernel_spmd(nc, in_maps, core_ids=list(range(N_CORES)), **kw)
    out = gather_out(res.results)
    if kw.get("trace"):
        return out, res
    return out
